# revision 19
# baseline (speedup 1.0000x reference)
"""Multi-head causal attention (B=2, T=2048, D=1024, H=16) on 8 NeuronCores.

Sharding: data-parallel over batch (cores 0-3 -> batch 0, cores 4-7 -> batch 1),
tensor-parallel over heads within each batch group (4 heads per core,
column-parallel w_q/w_k/w_v, row-parallel w_o). Each core returns a partial
[T, D] output for its batch; the host sums the 4 partials per batch.

Per-core kernel (all matmul inputs float32r, fp32 PSUM accumulation):
  phase A: Q^T,K^T = (w^T)^T-slices @ x^T  (heads on partitions), V = x @ w_v^T
           (tokens on partitions, ones-column augmented for the softmax denom)
  phase B: per head, per k-tile of 128 keys: S^T[k,q] = K_blk @ Q^T (causal
           q >= k only), e = exp(S^T/8) (ACT, PSUM->SBUF), diagonal-block
           causal mask multiply (DVE), then [V|1]^T @ e accumulated in PSUM
           -> unnormalized O^T rows 0-63 + denominator row 64.
           Normalize: reciprocal of denom, broadcast to 64 partitions via a
           K=1 matmul, elementwise multiply.
  phase C: out_partial = O^T-slices^T @ w_o-rows (K=64 per head, accumulated).
"""

import os
import sys
from contextlib import ExitStack

import numpy as np

import concourse.bacc as bacc
import concourse.bass as bass
import concourse.tile as tile
from concourse import mybir
from concourse.bass_utils import run_bass_kernel_spmd

B, T, D, H = 2, 2048, 1024, 16
HD = D // H  # 64
HL = 4  # heads per core
N_CORES = 8

F32 = mybir.dt.float32
F32R = mybir.dt.float32r

KT_D = D // 128  # 8 contraction tiles for the projections
TT = T // 128  # 16 token tiles
QW = 1024  # q window width in phase B
NCH = 512  # psum bank chunk

# tunable knobs (A/B testable); _get_module caches per (reps, knobs)
DEFAULT_KNOBS = dict(
    mask_on_pool=False,  # causal-mask multiply on GpSimd (else DVE)
    aps_bufs=4,         # phase A psum pool depth
    e_bufs=4,           # exp output SBUF pool depth
    den_on_act=True,    # denominator copy on ACT (else DVE)
    k_evac_act=True,    # K^T projection evac on ACT (else DVE)
    gps_bufs=4,         # uniform global PSUM pool slots (2 banks each)
)


def _emit(nc, reps=1, knobs=None):
    xt = nc.dram_tensor("xt", [D, T], F32R, kind="ExternalInput")
    wq = nc.dram_tensor("wq", [D, HL * HD], F32R, kind="ExternalInput")
    wk = nc.dram_tensor("wk", [D, HL * HD], F32R, kind="ExternalInput")
    wv = nc.dram_tensor("wv", [D, HL * HD], F32R, kind="ExternalInput")
    wo = nc.dram_tensor("wo", [128, 2 * D], F32R, kind="ExternalInput")
    mask = nc.dram_tensor("mask", [128, 128], F32R, kind="ExternalInput")
    vones = nc.dram_tensor("vones", [128, TT * HL], F32R, kind="ExternalInput")
    ones_b = nc.dram_tensor("ones_b", [1, HD], F32R, kind="ExternalInput")
    out = nc.dram_tensor("o", [T, D], F32, kind="ExternalOutput")

    xt_v = xt.ap().rearrange("(k p) m -> p k m", p=128)  # [128, 8, 2048]
    wq_v = wq.ap().rearrange("(k p) m -> p k m", p=128)  # [128, 8, 256]
    wk_v = wk.ap().rearrange("(k p) m -> p k m", p=128)
    wv_v = wv.ap().rearrange("(k p) m -> p k m", p=128)
    out_v = out.ap().rearrange("(t p) m -> t p m", p=128)  # [16, 128, 1024]

    views = (xt_v, wq_v, wk_v, wv_v, wo, mask, vones, ones_b, out_v)
    kb = dict(DEFAULT_KNOBS)
    if knobs:
        kb.update(knobs)
    with tile.TileContext(nc) as tc:
        if reps == 1:
            _body(nc, tc, views, kb)
        else:
            with tc.For_i(0, reps, 1):
                _body(nc, tc, views, kb)


def _body(nc, tc, views, kb):
    xt_v, wq_v, wk_v, wv_v, wo, mask, vones, ones_b, out_v = views
    mask_mul = nc.gpsimd.tensor_mul if kb["mask_on_pool"] else nc.vector.tensor_mul
    with ExitStack() as ctx:
        pers = ctx.enter_context(tc.tile_pool(name="pers", bufs=1))
        qk_pool = ctx.enter_context(tc.tile_pool(name="qk", bufs=1))
        ot_pool = ctx.enter_context(tc.tile_pool(name="ot", bufs=1))
        pha = ctx.enter_context(tc.tile_pool(name="pha", bufs=1))
        e_pool = ctx.enter_context(tc.tile_pool(name="e", bufs=kb["e_bufs"]))
        dn_pool = ctx.enter_context(tc.tile_pool(name="dn", bufs=2))
        tmp_pool = ctx.enter_context(tc.tile_pool(name="tmp", bufs=1))
        osb_pool = ctx.enter_context(tc.tile_pool(name="osb", bufs=2))
        # one uniform-slot PSUM pool shared by all phases: 4 slots x 2 banks
        gps = ctx.enter_context(
            tc.tile_pool(name="gps", bufs=kb["gps_bufs"], space="PSUM")
        )

        def ptile(shape, name):
            return gps.tile(shape, F32, tag="u", name=name,
                            padded_shape=[128, QW])

        # ---- input DMAs: first projection needs xt[kt0] + wq first ----
        xt_sb = pha.tile([128, KT_D, T], F32R, tag="xt")
        nc.sync.dma_start(xt_sb[:, 0, :], xt_v[:, 0, :])
        wq_sb = pha.tile([128, KT_D, HL * HD], F32R, tag="wq")
        nc.sync.dma_start(wq_sb[:], wq_v)
        for kt in range(1, KT_D):
            nc.sync.dma_start(xt_sb[:, kt, :], xt_v[:, kt, :])
        wk_sb = pha.tile([128, KT_D, HL * HD], F32R, tag="wk")
        nc.sync.dma_start(wk_sb[:], wk_v)
        wv_sb = pha.tile([128, KT_D, HL * HD], F32R, tag="wv")
        nc.sync.dma_start(wv_sb[:], wv_v)

        wo_sb = pers.tile([128, 2, D], F32R, tag="wo")
        nc.sync.dma_start(wo_sb[:], wo.ap().rearrange("p (g m) -> p g m", g=2))
        mask_sb = pers.tile([128, 128], F32R, tag="mask")
        nc.sync.dma_start(mask_sb[:], mask.ap())
        ones_sb = pers.tile([65, HD], F32R, tag="ones")
        nc.sync.dma_start(ones_sb[64:65, :], ones_b.ap())

        qT = qk_pool.tile([128, 2, T], F32R, tag="qT")  # [2 heads x 64, mg, T]
        kT = qk_pool.tile([128, 2, T], F32R, tag="kT")
        v_sb = qk_pool.tile([128, TT, HL, HD + 1], F32R, tag="v")
        ot = [
            ot_pool.tile([128, T], F32R, tag=f"ot{g}", name=f"ot{g}") for g in range(2)
        ]
        nc.sync.dma_start(v_sb[:, :, :, HD : HD + 1], vones.ap())

        # ---- phase A: projections ----
        def qk_proj(mg):
            for w_sb, dst, dve in (
                (wq_sb, qT, True),
                (wk_sb, kT, not kb["k_evac_act"]),
            ):
                for qc in range(T // NCH):
                    ps = ptile([128, NCH], "psq")
                    for kt in range(KT_D):
                        nc.tensor.matmul(
                            ps[:],
                            w_sb[:, kt, mg * 128 : (mg + 1) * 128],
                            xt_sb[:, kt, qc * NCH : (qc + 1) * NCH],
                            start=(kt == 0),
                            stop=(kt == KT_D - 1),
                        )
                    d = dst[:, mg, qc * NCH : (qc + 1) * NCH]
                    if dve:
                        nc.vector.tensor_copy(d, ps[:])
                    else:
                        nc.scalar.copy(d, ps[:])

        qk_proj(0)
        for tt in range(TT):
            ps = ptile([128, HL * HD], "psv")
            for kt in range(KT_D):
                nc.tensor.matmul(
                    ps[:],
                    xt_sb[:, kt, tt * 128 : (tt + 1) * 128],
                    wv_sb[:, kt, :],
                    start=(kt == 0),
                    stop=(kt == KT_D - 1),
                )
            nc.vector.tensor_copy(v_sb[:, tt, :, 0:HD], ps[:])
        qk_proj(1)

        # ---- phases B+C interleaved, qh-major ----
        def b_block(mg, qh):
            q0 = qh * QW
            ktn = (q0 + QW) // 128
            o_acc = [ptile([65, QW], f"oacc{hp}") for hp in range(2)]
            for kt in range(ktn):
                qs = max(0, kt * 128 - q0)
                segs = [(qs, NCH), (NCH, QW)] if qs < NCH else [(qs, QW)]
                for hp in range(2):
                    h = 2 * mg + hp
                    r0, r1 = hp * 64, hp * 64 + 64
                    st = ptile([128, QW], "st")
                    for s0, s1 in segs:
                        nc.tensor.matmul(
                            st[:, s0:s1],
                            kT[r0:r1, mg, kt * 128 : (kt + 1) * 128],
                            qT[r0:r1, mg, q0 + s0 : q0 + s1],
                            start=True,
                            stop=True,
                        )
                    e = e_pool.tile([128, QW], F32R, tag="e", name="e")
                    nc.scalar.activation(
                        e[:, qs:QW],
                        st[:, qs:QW],
                        mybir.ActivationFunctionType.Exp,
                        scale=0.125,
                    )
                    if kt * 128 >= q0:  # diagonal block: strict causal mask
                        mask_mul(
                            e[:, qs : qs + 128], e[:, qs : qs + 128], mask_sb[:]
                        )
                    for c in range(QW // NCH):
                        s0 = max(qs, c * NCH)
                        s1 = (c + 1) * NCH
                        if s0 >= s1:
                            continue
                        lastk = min(ktn - 1, (q0 + s1 - 1) // 128)
                        nc.tensor.matmul(
                            o_acc[hp][:, s0:s1],
                            v_sb[:, kt, h, :],
                            e[:, s0:s1],
                            start=(kt == 0),
                            stop=(kt == lastk),
                        )
            # ---- normalize (den = row 64 of o_acc) ----
            recips = []
            tmp = None
            for hp in range(2):
                den = dn_pool.tile([65, QW], F32, tag="den", name="den")
                if kb["den_on_act"]:
                    nc.scalar.copy(den[64:65, :], o_acc[hp][64:65, :])
                else:
                    nc.vector.tensor_copy(den[64:65, :], o_acc[hp][64:65, :])
                recip = dn_pool.tile([65, QW], F32R, tag="recip", name="rc")
                with nc.allow_low_precision(reason="fp32r recip"):
                    nc.vector.reciprocal(recip[64:65, :], den[64:65, :])
                recips.append(recip)
                if hp == 0:
                    nc.vector.tensor_copy(
                        ot[mg][0:64, q0 : q0 + QW], o_acc[hp][0:64, :]
                    )
                else:
                    tmp = tmp_pool.tile([64, QW], F32R, tag="tmp", name="tmp")
                    nc.vector.tensor_copy(tmp[:], o_acc[hp][0:64, :])
            for hp in range(2):
                rb = ptile([64, QW], "rb")
                for c in range(QW // NCH):
                    nc.tensor.matmul(
                        rb[:, c * NCH : (c + 1) * NCH],
                        ones_sb[64:65, :],
                        recips[hp][64:65, c * NCH : (c + 1) * NCH],
                        start=True,
                        stop=True,
                    )
                dst = ot[mg][0:64, q0 : q0 + QW] if hp == 0 else tmp[:]
                nc.vector.tensor_mul(dst, dst, rb[:])
            nc.sync.dma_start(ot[mg][64:128, q0 : q0 + QW], tmp[:])

        def c_tile(tt):
            ob = osb_pool.tile([128, D], F32, tag="ob", name="ob")
            for c in range(D // NCH):
                ps = ptile([128, NCH], "fp")
                for mg in range(2):
                    nc.tensor.matmul(
                        ps[:],
                        ot[mg][:, tt * 128 : (tt + 1) * 128],
                        wo_sb[:, mg, c * NCH : (c + 1) * NCH],
                        start=(mg == 0),
                        stop=(mg == 1),
                    )
                d = ob[:, c * NCH : (c + 1) * NCH]
                if c % 2 == 0:
                    nc.vector.tensor_copy(d, ps[:])
                else:
                    nc.scalar.copy(d, ps[:])
            nc.sync.dma_start(out_v[tt], ob[:])

        for qh in range(T // QW):
            for mg in range(2):
                b_block(mg, qh)
            for tt in range(qh * (QW // 128), (qh + 1) * (QW // 128)):
                c_tile(tt)


_NC_CACHE = {}


def _get_module(reps=1, knobs=None):
    key = (reps, tuple(sorted((knobs or {}).items())))
    if key not in _NC_CACHE:
        nc = bacc.Bacc("TRN2", target_bir_lowering=False, debug=False)
        _emit(nc, reps=reps, knobs=knobs)
        nc.compile()
        _NC_CACHE[key] = nc
    return _NC_CACHE[key]


def _in_maps(x, w_q, w_k, w_v, w_o):
    """Build the 8 per-core input dicts from the full-problem arrays."""
    mask = np.triu(np.ones((128, 128), dtype=np.float32))
    vones = np.ones((128, TT * HL), dtype=np.float32)
    ones_b = np.ones((1, HD), dtype=np.float32)
    maps = []
    for c in range(N_CORES):
        b, g = c // 4, c % 4
        hs = g * HL * HD  # first output-dim of this core's heads
        sl = slice(hs, hs + HL * HD)
        wo_g = np.ascontiguousarray(
            w_o[:, sl].T.reshape(2, 128, D).transpose(1, 0, 2).reshape(128, 2 * D)
        )
        maps.append(
            {
                "xt": np.ascontiguousarray(x[b].T),
                "wq": np.ascontiguousarray(w_q[sl, :].T),
                "wk": np.ascontiguousarray(w_k[sl, :].T),
                "wv": np.ascontiguousarray(w_v[sl, :].T),
                "wo": wo_g,
                "mask": mask,
                "vones": vones,
                "ones_b": ones_b,
            }
        )
    return maps


def _run(inputs, trace=False, reps=1, knobs=None, **kw):
    nc = _get_module(reps, knobs)
    maps = _in_maps(
        np.asarray(inputs["x"], dtype=np.float32),
        np.asarray(inputs["w_q"], dtype=np.float32),
        np.asarray(inputs["w_k"], dtype=np.float32),
        np.asarray(inputs["w_v"], dtype=np.float32),
        np.asarray(inputs["w_o"], dtype=np.float32),
    )
    res = run_bass_kernel_spmd(nc, maps, list(range(N_CORES)), trace=trace, **kw)
    parts = [res.results[c]["o"] for c in range(N_CORES)]
    out = np.stack(
        [
            parts[0] + parts[1] + parts[2] + parts[3],
            parts[4] + parts[5] + parts[6] + parts[7],
        ]
    ).astype(np.float32)
    return out, res


def kernel(**inputs):
    out, _ = _run(inputs)
    return out


# ---------------------------------------------------------------------------
# timing helpers (test.py only): cached jit runner, device-resident inputs,
# on-device zero output buffers. Mirrors bass2jax.run_bass_via_pjrt exactly
# (incl. donation) but jits once so per-sample wall is dispatch + exec.
_RUNNER_CACHE = {}


def _make_runner(reps, knobs=None):
    key = (reps, tuple(sorted((knobs or {}).items())))
    if key in _RUNNER_CACHE:
        return _RUNNER_CACHE[key]
    import jax
    from jax.sharding import Mesh, NamedSharding, PartitionSpec
    from jax.experimental.shard_map import shard_map
    from concourse.bass2jax import (
        _bass_exec_p,
        install_neuronx_cc_hook,
        partition_id_tensor,
    )

    nc = _get_module(reps, knobs)
    install_neuronx_cc_hook()
    pname = nc.partition_id_tensor.name if nc.partition_id_tensor else None
    in_names, out_names, out_avals = [], [], []
    for alloc in nc.m.functions[0].allocations:
        if not isinstance(alloc, mybir.MemoryLocationSet):
            continue
        name = alloc.memorylocations[0].name
        if alloc.kind == "ExternalInput":
            if name != pname:
                in_names.append(name)
        elif alloc.kind == "ExternalOutput":
            out_names.append(name)
            out_avals.append(
                jax.core.ShapedArray(tuple(alloc.tensor_shape), mybir.dt.np(alloc.dtype))
            )
    n_params = len(in_names)
    bind_names = in_names + out_names + ([pname] if pname else [])

    def _bd(*args):
        operands = list(args)
        if pname:
            operands.append(partition_id_tensor())
        return tuple(
            _bass_exec_p.bind(
                *operands,
                out_avals=tuple(out_avals),
                in_names=tuple(bind_names),
                out_names=tuple(out_names),
                lowering_input_output_aliases=(),
                sim_require_finite=True,
                sim_require_nnan=True,
                nc=nc,
            )
        )

    devices = jax.devices()[:N_CORES]
    mesh = Mesh(np.asarray(devices), ("core",))
    nspec = n_params + len(out_names)
    fn = jax.jit(
        shard_map(
            _bd,
            mesh=mesh,
            in_specs=(PartitionSpec("core"),) * nspec,
            out_specs=(PartitionSpec("core"),) * len(out_names),
            check_rep=False,
        ),
        donate_argnums=tuple(range(n_params, n_params + len(out_names))),
        keep_unused=True,
    )
    shard = NamedSharding(mesh, PartitionSpec("core"))
    zfn = jax.jit(
        lambda: tuple(
            jax.numpy.zeros((N_CORES * a.shape[0], *a.shape[1:]), a.dtype)
            for a in out_avals
        ),
        out_shardings=(shard,) * len(out_names),
    )
    _RUNNER_CACHE[key] = (fn, zfn, in_names, out_names, out_avals, shard)
    return _RUNNER_CACHE[key]


def _time_exec(inputs, reps, nsamples=8, knobs=None):
    """Return (min wall seconds per call, last output array [8,T,D])."""
    import time as _time
    import jax

    fn, zfn, in_names, out_names, out_avals, shard = _make_runner(reps, knobs)
    maps = _in_maps(
        np.asarray(inputs["x"], dtype=np.float32),
        np.asarray(inputs["w_q"], dtype=np.float32),
        np.asarray(inputs["w_k"], dtype=np.float32),
        np.asarray(inputs["w_v"], dtype=np.float32),
        np.asarray(inputs["w_o"], dtype=np.float32),
    )
    dev_in = [
        jax.device_put(
            np.concatenate([maps[c][n] for c in range(N_CORES)], axis=0), shard
        )
        for n in in_names
    ]
    out = fn(*dev_in, *zfn())  # warmup (compile + first exec)
    jax.block_until_ready(out)
    walls = []
    for _ in range(nsamples):
        zeros = zfn()
        jax.block_until_ready(zeros)
        t0 = _time.perf_counter()
        out = fn(*dev_in, *zeros)
        jax.block_until_ready(out)
        walls.append(_time.perf_counter() - t0)
    o = np.asarray(out[0]).reshape(N_CORES, T, D)
    return min(walls), walls, o


if __name__ == "__main__":
    rng = np.random.default_rng(0)
    ins = {
        "x": rng.standard_normal((B, T, D), dtype=np.float32),
        "w_q": (rng.standard_normal((D, D)) * 0.02).astype(np.float32),
        "w_k": (rng.standard_normal((D, D)) * 0.02).astype(np.float32),
        "w_v": (rng.standard_normal((D, D)) * 0.02).astype(np.float32),
        "w_o": (rng.standard_normal((D, D)) * 0.02).astype(np.float32),
    }
    out = kernel(**ins)
    print("ok", out.shape, out.dtype)
